# revision 5
# baseline (speedup 1.0000x reference)
"""Trainium2 Bass kernel v3: baseline fp16 engine scheme + exact triangular
trim of the (block, step) iteration space.

Trim (exact, zero error):
  - dead-code: v[n] at step s only reaches the head if s <= steps-9+n
  - zero-prop: with block_b == 0, v[n] stays exactly 0 until step n+1
  => 210 of 300 block-steps remain at steps=30 (30% less PE work).

Schedule: per step, active blocks descending, paired into units of <=2.
Pipeline: phase_a(i+1) emitted before phase_b(i); where unit i+1 reads a v
written by unit i, a bubble (None unit) is inserted so emission order stays
correct (Tile gives sequential semantics per tile in emission order).
"""

import numpy as np

import concourse.bass as bass
import concourse.bacc as bacc
import concourse.mybir as mybir
from concourse.bass_utils import run_bass_kernel_spmd
from concourse.tile import TileContext

F32 = mybir.dt.float32
F16 = mybir.dt.float16

B, DIN, H, DOUT, NB = 1024, 512, 1024, 512, 10
NCORES = 8
BL = B // NCORES  # 128
KH = H // 128     # 8
KD = DIN // 128   # 4
KO = DOUT // 128  # 4
INNER = 5
Tanh = mybir.ActivationFunctionType.Tanh
Copy = mybir.ActivationFunctionType.Copy
Ident = mybir.ActivationFunctionType.Identity
MULT = mybir.AluOpType.mult
ADD = mybir.AluOpType.add


def build_units(steps: int, with_bias: bool):
    """Unit list: each entry is a list of (s, n) block-steps (len 1-2) or
    None (pipeline bubble)."""
    units = []
    for s in range(1, steps + 1):
        ns = [n for n in range(NB - 1, -1, -1)
              if s <= steps - (NB - 1) + n and (with_bias or s >= n + 1)]
        for i in range(0, len(ns), 2):
            units.append([(s, n) for n in ns[i:i + 2]])

    def writes(u):
        return {n for (_, n) in u} if u else set()

    def reads(u):
        r = set()
        if u:
            for (_, n) in u:
                r.add(n)
                if n > 0:
                    r.add(n - 1)
        return r

    out = []
    for u in units:
        prev = out[-1] if out else None
        if prev is not None and (writes(prev) & reads(u)):
            out.append(None)
        out.append(u)
    return out


def build_nc(steps: int, with_bias: bool = False):
    nc = bacc.Bacc(None, target_bir_lowering=False)
    xT = nc.dram_tensor("xT", [128, KD * BL], F16, kind="ExternalInput")
    embWT = nc.dram_tensor("embWT", [128, KD * H], F16, kind="ExternalInput")
    embB = nc.dram_tensor("embB", [128, KH], F32, kind="ExternalInput")
    Wab = nc.dram_tensor("Wab", [2, NB, 128, KH * H], F16, kind="ExternalInput")
    bT = nc.dram_tensor("bT", [128, NB * KH], F32, kind="ExternalInput")
    headWT = nc.dram_tensor("headWT", [128, KH * DOUT], F16, kind="ExternalInput")
    headB = nc.dram_tensor("headB", [128, KO], F32, kind="ExternalInput")
    outT = nc.dram_tensor("outT", [128, KO * BL], F32, kind="ExternalOutput")

    units = build_units(steps, with_bias)
    nreal = sum(1 for u in units if u)

    with TileContext(nc) as tc:
        with (
            tc.tile_pool(name="const", bufs=1) as cpool,
            tc.tile_pool(name="state", bufs=1) as spool,
            tc.tile_pool(name="wts", bufs=4) as wpool,
            tc.tile_pool(name="work", bufs=2) as kpool,
            tc.tile_pool(name="psum", bufs=1, space="PSUM") as ppool,
        ):
            xT_sb = cpool.tile([128, KD * BL], F16, tag="xt", bufs=1)
            embWT_sb = cpool.tile([128, KD * H], F16, tag="embwt", bufs=1)
            embB_sb = cpool.tile([128, KH], F32, tag="embb", bufs=1)
            bT_sb = cpool.tile([128, NB * KH], F32, tag="bt", bufs=1)
            headWT_sb = cpool.tile([128, KH * DOUT], F16, tag="hwt", bufs=1)
            headB_sb = cpool.tile([128, KO], F32, tag="hb", bufs=1)

            nc.sync.dma_start(xT_sb[:], xT[:])
            nc.sync.dma_start(embWT_sb[:], embWT[:])
            nc.sync.dma_start(embB_sb[:], embB[:])
            if with_bias:
                nc.sync.dma_start(bT_sb[:], bT[:])

            v = [spool.tile([128, H], F32, tag=f"v{n}", bufs=1, name=f"v{n}")
                 for n in range(NB)]
            xemb = spool.tile([128, H], F32, tag="xemb", bufs=1)
            for n in range(NB):
                if n % 2 == 0:
                    nc.vector.memset(v[n][:], 0.0)
                else:
                    nc.gpsimd.memset(v[n][:], 0.0)

            bfull = None
            if with_bias:
                # fp16 bias tensors (SBUF budget); bias-add error ~2^-11 on c
                bfull = [spool.tile([128, H], F16, tag=f"bf{n}", bufs=1,
                                    name=f"bf{n}") for n in range(NB)]
                zed = spool.tile([128, 128], F32, tag="zed", bufs=1)
                nc.vector.memset(zed[:], 0.0)
                for n in range(NB):
                    for m in range(KH):
                        nc.scalar.activation(
                            bfull[n][:, m * 128:(m + 1) * 128], zed[:], Ident,
                            bias=bT_sb[:, n * KH + m:n * KH + m + 1], scale=0.0)

            # psum banks: [set][pair-position][half]
            pbank = [[[ppool.tile([128, 512], F32, tag=f"ps{s}{p}{h}", bufs=1,
                                  name=f"ps{s}{p}{h}")
                       for h in range(2)] for p in range(2)] for s in range(2)]

            def emit_warm(bank):
                # Set every psum element's has_written bit via a full-bank
                # start=True matmul (values are garbage; prefills replace
                # them). Needed so later start=False matmuls ACCUMULATE onto
                # Act/DVE-prefilled values instead of overwriting them —
                # only TensorE matmuls set has_written.
                nc.tensor.matmul(
                    bank[:], xT_sb[:, 0:128], xT_sb[:, 0:KD * BL],
                    start=True, stop=True, skip_group_check=True,
                )

            def emit_embed():
                for m in range(KH):
                    pe = pbank[1][1][1][:, (m % 4) * 128:(m % 4 + 1) * 128]
                    for k in range(KD):
                        nc.tensor.matmul(
                            pe,
                            embWT_sb[:, k * H + m * 128:k * H + (m + 1) * 128],
                            xT_sb[:, k * BL:(k + 1) * BL],
                            start=(k == 0), stop=(k == KD - 1),
                        )
                    nc.scalar.activation(
                        xemb[:, m * 128:(m + 1) * 128], pe, Ident,
                        bias=embB_sb[:, m:m + 1], scale=1.0,
                    )

            state = {}   # unit idx -> {n: [w, c, vh, u]}
            sets = {}    # unit idx -> psum set

            def emit_phase_a(i, fast=False):
                u_list = units[i]
                if u_list is None:
                    return
                S = sets[i]
                ust = {}
                wb = 3 if with_bias else 4  # SBUF budget in bias mode
                for pi, (s, n) in enumerate(u_list):
                    par = (s + 1) % 2  # s=1 -> Wa (matches baseline order)
                    w = []
                    for k in range(KH):
                        wk = wpool.tile([128, H], F16, tag=f"w{k}", bufs=wb,
                                        name=f"w{k}")
                        nc.sync.dma_start(
                            wk[:], Wab[par, n, :, k * H:(k + 1) * H])
                        w.append(wk)
                    binT = xemb if n == 0 else v[n - 1]
                    c = kpool.tile([128, H], F32, tag="c", bufs=4, name="c")
                    if with_bias:
                        nc.gpsimd.tensor_tensor(c[:], v[n][:], binT[:], ADD)
                        nc.gpsimd.tensor_tensor(c[:], c[:], bfull[n][:], ADD)
                    elif fast:
                        # post-bubble units: c-add is latency-critical; DVE
                        # (1.07us) beats gpsimd Add at 0.42 eff (2.03us)
                        nc.vector.scalar_tensor_tensor(
                            c[:], v[n][:], 1.0, binT[:], MULT, ADD)
                    else:
                        nc.gpsimd.tensor_tensor(c[:], v[n][:], binT[:], ADD)
                    vh = kpool.tile([128, H], F32, tag="vh", bufs=4, name="vh")
                    nc.gpsimd.tensor_scalar_mul(vh[:], v[n][:], 0.5)
                    u = kpool.tile([128, H], F16, tag="u", bufs=6, name="u")
                    nc.scalar.activation(u[:], c[:], Tanh, bias=0.0, scale=1.0)
                    nc.scalar.activation(pbank[S][pi][0][:], c[:, 0:512],
                                         Copy, scale=1.0)
                    nc.vector.tensor_copy(pbank[S][pi][1][:], c[:, 512:1024])
                    ust[n] = [w, c, vh, u]
                state[i] = ust

            def emit_phase_b(i):
                u_list = units[i]
                if u_list is None:
                    return
                S = sets[i]
                ust = state.pop(i)
                order = [n for (_, n) in u_list]
                for j in range(1, INNER):
                    last = j == INNER - 1
                    for pi, n in enumerate(order):
                        w, c, vh, u = ust[n]
                        t = kpool.tile([128, H], F16, tag="t", bufs=4,
                                       name="t")
                        for h in range(2):
                            bank = pbank[S][pi][h]
                            for m in range(4):
                                mg = h * 4 + m
                                for k in range(KH):
                                    nc.tensor.matmul(
                                        bank[:, m * 128:(m + 1) * 128],
                                        w[k][:, mg * 128:(mg + 1) * 128],
                                        u[:, k * 128:(k + 1) * 128],
                                        start=False, stop=(k == KH - 1),
                                        skip_group_check=True,
                                    )
                            nc.scalar.activation(
                                t[:, h * 512:(h + 1) * 512], bank[:], Tanh,
                                bias=0.0, scale=1.0)
                            if not last:
                                if h == 0:
                                    nc.scalar.activation(
                                        bank[:], c[:, 0:512], Copy, scale=1.0)
                                else:
                                    nc.vector.tensor_copy(
                                        bank[:], c[:, 512:1024])
                        if not last:
                            un = kpool.tile([128, H], F16, tag="u", bufs=6,
                                            name="un")
                            nc.vector.scalar_tensor_tensor(
                                un[:], u[:], 0.5, t[:], MULT, ADD)
                            ust[n][3] = un
                        else:
                            u5 = kpool.tile([128, H], F16, tag="u5", bufs=2,
                                            name="u5")
                            nc.vector.scalar_tensor_tensor(
                                u5[:], u[:], 0.5, t[:], MULT, ADD)
                            nc.vector.scalar_tensor_tensor(
                                v[n][:], u5[:], 0.25, vh[:], MULT, ADD)

            def emit_head(S):
                nc.sync.dma_start(headWT_sb[:], headWT[:])
                nc.sync.dma_start(headB_sb[:], headB[:])
                outsb = kpool.tile([128, KO * BL], F32, tag="outsb", bufs=1)
                v9h = kpool.tile([128, H], F16, tag="v9h", bufs=1)
                nc.vector.tensor_copy(v9h[:], v[NB - 1][:])
                for m in range(KO):
                    ph = pbank[S][0][0][:, m * 128:(m + 1) * 128]
                    for k in range(KH):
                        nc.tensor.matmul(
                            ph,
                            headWT_sb[:, k * DOUT + m * 128:k * DOUT + (m + 1) * 128],
                            v9h[:, k * 128:(k + 1) * 128],
                            start=(k == 0), stop=(k == KH - 1),
                        )
                    nc.scalar.activation(
                        outsb[:, m * BL:(m + 1) * BL], ph, Ident,
                        bias=headB_sb[:, m:m + 1], scale=1.0,
                    )
                nc.sync.dma_start(outT[:], outsb[:])

            # assign psum sets by real-unit parity
            r = 0
            for i, u_list in enumerate(units):
                if u_list is not None:
                    sets[i] = r % 2
                    r += 1

            for S in range(2):
                for p in range(2):
                    for h in range(2):
                        emit_warm(pbank[S][p][h])
            emit_embed()
            # embed's start=True groups cleared this bank's has_written bits
            # and only re-set the last slice; re-warm before unit use.
            emit_warm(pbank[1][1][1])
            if units:
                emit_phase_a(0, fast=True)
            for i in range(len(units)):
                if i + 1 < len(units):
                    emit_phase_a(i + 1, fast=(units[i] is None))
                emit_phase_b(i)
            emit_head(nreal % 2)

    nc.compile()
    return nc


def _tile_k(a):
    """[K, M] -> [128, (K//128)*M] laid out (k_lo, k_hi, m)."""
    K, M = a.shape
    return np.ascontiguousarray(
        a.reshape(K // 128, 128, M).transpose(1, 0, 2).reshape(128, (K // 128) * M)
    )


def kernel(**inputs) -> np.ndarray:
    x = np.asarray(inputs["x"], np.float32)
    embed_W = np.asarray(inputs["embed_W"], np.float32)
    embed_b = np.asarray(inputs["embed_b"], np.float32)
    block_W = np.asarray(inputs["block_W"], np.float32)
    block_b = np.asarray(inputs["block_b"], np.float32)
    head_W = np.asarray(inputs["head_W"], np.float32)
    head_b = np.asarray(inputs["head_b"], np.float32)
    steps = int(np.asarray(inputs["steps"]))
    with_bias = bool(np.any(block_b))

    embWT = _tile_k(embed_W.T).astype(np.float16)
    headWT = _tile_k(head_W.T).astype(np.float16)
    Wt = block_W.transpose(0, 2, 1) * np.float32(0.5)
    Wa = Wt.astype(np.float16)
    Wb = (2.0 * Wt - Wa.astype(np.float32)).astype(np.float16)
    Wab = np.stack(
        [
            np.stack([_tile_k(Wa[n]) for n in range(NB)]),
            np.stack([_tile_k(Wb[n]) for n in range(NB)]),
        ]
    )
    embB = np.ascontiguousarray(embed_b.reshape(KH, 128).T)
    bT = np.ascontiguousarray(
        block_b.reshape(NB, KH, 128).transpose(2, 0, 1).reshape(128, NB * KH)
    )
    headB = np.ascontiguousarray(head_b.reshape(KO, 128).T)

    in_maps = []
    for ci in range(NCORES):
        xTl = _tile_k(np.ascontiguousarray(x[ci * BL:(ci + 1) * BL].T)).astype(np.float16)
        in_maps.append(
            dict(xT=xTl, embWT=embWT, embB=embB, Wab=Wab, bT=bT,
                 headWT=headWT, headB=headB)
        )

    nc = build_nc(steps, with_bias)
    res = run_bass_kernel_spmd(nc, in_maps, core_ids=list(range(NCORES)))

    out = np.empty((B, DOUT), np.float32)
    for ci in range(NCORES):
        oT = res.results[ci]["outT"]
        out[ci * BL:(ci + 1) * BL] = (
            oT.reshape(128, KO, BL).transpose(2, 1, 0).reshape(BL, DOUT)
        )
    return out


# revision 6
# speedup vs baseline: 1.0008x; 1.0008x over previous
"""Trainium2 Bass kernel v3: baseline fp16 engine scheme + exact triangular
trim of the (block, step) iteration space.

Trim (exact, zero error):
  - dead-code: v[n] at step s only reaches the head if s <= steps-9+n
  - zero-prop: with block_b == 0, v[n] stays exactly 0 until step n+1
  => 210 of 300 block-steps remain at steps=30 (30% less PE work).

Schedule: per step, active blocks descending, paired into units of <=2.
Pipeline: phase_a(i+1) emitted before phase_b(i); where unit i+1 reads a v
written by unit i, a bubble (None unit) is inserted so emission order stays
correct (Tile gives sequential semantics per tile in emission order).
"""

import numpy as np

import concourse.bass as bass
import concourse.bacc as bacc
import concourse.mybir as mybir
from concourse.bass_utils import run_bass_kernel_spmd
from concourse.tile import TileContext

F32 = mybir.dt.float32
F16 = mybir.dt.float16

B, DIN, H, DOUT, NB = 1024, 512, 1024, 512, 10
NCORES = 8
BL = B // NCORES  # 128
KH = H // 128     # 8
KD = DIN // 128   # 4
KO = DOUT // 128  # 4
INNER = 5
Tanh = mybir.ActivationFunctionType.Tanh
Copy = mybir.ActivationFunctionType.Copy
Ident = mybir.ActivationFunctionType.Identity
MULT = mybir.AluOpType.mult
ADD = mybir.AluOpType.add


def build_units(steps: int, with_bias: bool):
    """Unit list: each entry is a list of (s, n) block-steps (len 1-2) or
    None (pipeline bubble)."""
    units = []
    for s in range(1, steps + 1):
        ns = [n for n in range(NB - 1, -1, -1)
              if s <= steps - (NB - 1) + n and (with_bias or s >= n + 1)]
        for i in range(0, len(ns), 2):
            units.append([(s, n) for n in ns[i:i + 2]])

    def writes(u):
        return {n for (_, n) in u} if u else set()

    def reads(u):
        r = set()
        if u:
            for (_, n) in u:
                r.add(n)
                if n > 0:
                    r.add(n - 1)
        return r

    out = []
    for u in units:
        prev = out[-1] if out else None
        if prev is not None and (writes(prev) & reads(u)):
            out.append(None)
        out.append(u)
    return out


def build_nc(steps: int, with_bias: bool = False):
    nc = bacc.Bacc(None, target_bir_lowering=False)
    xT = nc.dram_tensor("xT", [128, KD * BL], F16, kind="ExternalInput")
    embWT = nc.dram_tensor("embWT", [128, KD * H], F16, kind="ExternalInput")
    embB = nc.dram_tensor("embB", [128, KH], F32, kind="ExternalInput")
    Wab = nc.dram_tensor("Wab", [2, NB, 128, KH * H], F16, kind="ExternalInput")
    bT = nc.dram_tensor("bT", [128, NB * KH], F32, kind="ExternalInput")
    headWT = nc.dram_tensor("headWT", [128, KH * DOUT], F16, kind="ExternalInput")
    headB = nc.dram_tensor("headB", [128, KO], F32, kind="ExternalInput")
    outT = nc.dram_tensor("outT", [128, KO * BL], F32, kind="ExternalOutput")

    units = build_units(steps, with_bias)
    nreal = sum(1 for u in units if u)

    with TileContext(nc) as tc:
        with (
            tc.tile_pool(name="const", bufs=1) as cpool,
            tc.tile_pool(name="state", bufs=1) as spool,
            tc.tile_pool(name="wts", bufs=4) as wpool,
            tc.tile_pool(name="work", bufs=2) as kpool,
            tc.tile_pool(name="psum", bufs=1, space="PSUM") as ppool,
        ):
            xT_sb = cpool.tile([128, KD * BL], F16, tag="xt", bufs=1)
            embWT_sb = cpool.tile([128, KD * H], F16, tag="embwt", bufs=1)
            embB_sb = cpool.tile([128, KH], F32, tag="embb", bufs=1)
            bT_sb = cpool.tile([128, NB * KH], F32, tag="bt", bufs=1)
            headWT_sb = cpool.tile([128, KH * DOUT], F16, tag="hwt", bufs=1)
            headB_sb = cpool.tile([128, KO], F32, tag="hb", bufs=1)

            nc.sync.dma_start(xT_sb[:], xT[:])
            nc.sync.dma_start(embWT_sb[:], embWT[:])
            nc.sync.dma_start(embB_sb[:], embB[:])
            if with_bias:
                nc.sync.dma_start(bT_sb[:], bT[:])

            v = [spool.tile([128, H], F32, tag=f"v{n}", bufs=1, name=f"v{n}")
                 for n in range(NB)]
            xemb = spool.tile([128, H], F32, tag="xemb", bufs=1)
            for n in range(NB):
                if n % 2 == 0:
                    nc.vector.memset(v[n][:], 0.0)
                else:
                    nc.gpsimd.memset(v[n][:], 0.0)

            bfull = None
            if with_bias:
                # fp16 bias tensors (SBUF budget); bias-add error ~2^-11 on c
                bfull = [spool.tile([128, H], F16, tag=f"bf{n}", bufs=1,
                                    name=f"bf{n}") for n in range(NB)]
                zed = spool.tile([128, 128], F32, tag="zed", bufs=1)
                nc.vector.memset(zed[:], 0.0)
                for n in range(NB):
                    for m in range(KH):
                        nc.scalar.activation(
                            bfull[n][:, m * 128:(m + 1) * 128], zed[:], Ident,
                            bias=bT_sb[:, n * KH + m:n * KH + m + 1], scale=0.0)

            # psum banks: [set][pair-position][half]
            pbank = [[[ppool.tile([128, 512], F32, tag=f"ps{s}{p}{h}", bufs=1,
                                  name=f"ps{s}{p}{h}")
                       for h in range(2)] for p in range(2)] for s in range(2)]

            def emit_warm(bank):
                # Set every psum element's has_written bit via a full-bank
                # start=True matmul (values are garbage; prefills replace
                # them). Needed so later start=False matmuls ACCUMULATE onto
                # Act/DVE-prefilled values instead of overwriting them —
                # only TensorE matmuls set has_written.
                nc.tensor.matmul(
                    bank[:], xT_sb[:, 0:128], xT_sb[:, 0:KD * BL],
                    start=True, stop=True, skip_group_check=True,
                )

            def emit_embed():
                for m in range(KH):
                    pe = pbank[1][1][1][:, (m % 4) * 128:(m % 4 + 1) * 128]
                    for k in range(KD):
                        nc.tensor.matmul(
                            pe,
                            embWT_sb[:, k * H + m * 128:k * H + (m + 1) * 128],
                            xT_sb[:, k * BL:(k + 1) * BL],
                            start=(k == 0), stop=(k == KD - 1),
                        )
                    nc.scalar.activation(
                        xemb[:, m * 128:(m + 1) * 128], pe, Ident,
                        bias=embB_sb[:, m:m + 1], scale=1.0,
                    )

            state = {}   # unit idx -> {n: [w, c, vh, u]}
            sets = {}    # unit idx -> psum set

            def emit_phase_a(i, fast=False):
                u_list = units[i]
                if u_list is None:
                    return
                S = sets[i]
                ust = {}
                wb = 3 if with_bias else 4  # SBUF budget in bias mode
                for pi, (s, n) in enumerate(u_list):
                    par = (s + 1) % 2  # s=1 -> Wa (matches baseline order)
                    w = []
                    for k in range(KH):
                        wk = wpool.tile([128, H], F16, tag=f"w{k}", bufs=wb,
                                        name=f"w{k}")
                        nc.sync.dma_start(
                            wk[:], Wab[par, n, :, k * H:(k + 1) * H])
                        w.append(wk)
                    binT = xemb if n == 0 else v[n - 1]
                    c = kpool.tile([128, H], F32, tag="c", bufs=4, name="c")
                    if with_bias:
                        nc.gpsimd.tensor_tensor(c[:], v[n][:], binT[:], ADD)
                        nc.gpsimd.tensor_tensor(c[:], c[:], bfull[n][:], ADD)
                    elif fast:
                        # post-bubble units: c-add is latency-critical; split
                        # halves across DVE (0.54us) and gpsimd (1.02us)
                        nc.vector.scalar_tensor_tensor(
                            c[:, 0:512], v[n][:, 0:512], 1.0,
                            binT[:, 0:512], MULT, ADD)
                        nc.gpsimd.tensor_tensor(
                            c[:, 512:1024], v[n][:, 512:1024],
                            binT[:, 512:1024], ADD)
                    else:
                        nc.gpsimd.tensor_tensor(c[:], v[n][:], binT[:], ADD)
                    vh = kpool.tile([128, H], F32, tag="vh", bufs=4, name="vh")
                    nc.gpsimd.tensor_scalar_mul(vh[:], v[n][:], 0.5)
                    u = kpool.tile([128, H], F16, tag="u", bufs=6, name="u")
                    nc.scalar.activation(u[:], c[:], Tanh, bias=0.0, scale=1.0)
                    nc.scalar.activation(pbank[S][pi][0][:], c[:, 0:512],
                                         Copy, scale=1.0)
                    nc.vector.tensor_copy(pbank[S][pi][1][:], c[:, 512:1024])
                    ust[n] = [w, c, vh, u]
                state[i] = ust

            def emit_phase_b(i):
                u_list = units[i]
                if u_list is None:
                    return
                S = sets[i]
                ust = state.pop(i)
                order = [n for (_, n) in u_list]
                for j in range(1, INNER):
                    last = j == INNER - 1
                    for pi, n in enumerate(order):
                        w, c, vh, u = ust[n]
                        t = kpool.tile([128, H], F16, tag="t", bufs=4,
                                       name="t")
                        for h in range(2):
                            bank = pbank[S][pi][h]
                            for m in range(4):
                                mg = h * 4 + m
                                for k in range(KH):
                                    nc.tensor.matmul(
                                        bank[:, m * 128:(m + 1) * 128],
                                        w[k][:, mg * 128:(mg + 1) * 128],
                                        u[:, k * 128:(k + 1) * 128],
                                        start=False, stop=(k == KH - 1),
                                        skip_group_check=True,
                                    )
                            nc.scalar.activation(
                                t[:, h * 512:(h + 1) * 512], bank[:], Tanh,
                                bias=0.0, scale=1.0)
                            if not last:
                                if h == 0:
                                    nc.scalar.activation(
                                        bank[:], c[:, 0:512], Copy, scale=1.0)
                                else:
                                    nc.vector.tensor_copy(
                                        bank[:], c[:, 512:1024])
                        if not last:
                            un = kpool.tile([128, H], F16, tag="u", bufs=6,
                                            name="un")
                            nc.vector.scalar_tensor_tensor(
                                un[:], u[:], 0.5, t[:], MULT, ADD)
                            ust[n][3] = un
                        else:
                            u5 = kpool.tile([128, H], F16, tag="u5", bufs=2,
                                            name="u5")
                            nc.vector.scalar_tensor_tensor(
                                u5[:], u[:], 0.5, t[:], MULT, ADD)
                            nc.vector.scalar_tensor_tensor(
                                v[n][:], u5[:], 0.25, vh[:], MULT, ADD)

            def emit_head(S):
                nc.sync.dma_start(headWT_sb[:], headWT[:])
                nc.sync.dma_start(headB_sb[:], headB[:])
                outsb = kpool.tile([128, KO * BL], F32, tag="outsb", bufs=1)
                v9h = kpool.tile([128, H], F16, tag="v9h", bufs=1)
                nc.vector.tensor_copy(v9h[:], v[NB - 1][:])
                for m in range(KO):
                    ph = pbank[S][0][0][:, m * 128:(m + 1) * 128]
                    for k in range(KH):
                        nc.tensor.matmul(
                            ph,
                            headWT_sb[:, k * DOUT + m * 128:k * DOUT + (m + 1) * 128],
                            v9h[:, k * 128:(k + 1) * 128],
                            start=(k == 0), stop=(k == KH - 1),
                        )
                    nc.scalar.activation(
                        outsb[:, m * BL:(m + 1) * BL], ph, Ident,
                        bias=headB_sb[:, m:m + 1], scale=1.0,
                    )
                nc.sync.dma_start(outT[:], outsb[:])

            # assign psum sets by real-unit parity
            r = 0
            for i, u_list in enumerate(units):
                if u_list is not None:
                    sets[i] = r % 2
                    r += 1

            for S in range(2):
                for p in range(2):
                    for h in range(2):
                        emit_warm(pbank[S][p][h])
            emit_embed()
            # embed's start=True groups cleared this bank's has_written bits
            # and only re-set the last slice; re-warm before unit use.
            emit_warm(pbank[1][1][1])
            if units:
                emit_phase_a(0, fast=True)
            for i in range(len(units)):
                if i + 1 < len(units):
                    emit_phase_a(i + 1, fast=(units[i] is None))
                emit_phase_b(i)
            emit_head(nreal % 2)

    nc.compile()
    return nc


def _tile_k(a):
    """[K, M] -> [128, (K//128)*M] laid out (k_lo, k_hi, m)."""
    K, M = a.shape
    return np.ascontiguousarray(
        a.reshape(K // 128, 128, M).transpose(1, 0, 2).reshape(128, (K // 128) * M)
    )


def kernel(**inputs) -> np.ndarray:
    x = np.asarray(inputs["x"], np.float32)
    embed_W = np.asarray(inputs["embed_W"], np.float32)
    embed_b = np.asarray(inputs["embed_b"], np.float32)
    block_W = np.asarray(inputs["block_W"], np.float32)
    block_b = np.asarray(inputs["block_b"], np.float32)
    head_W = np.asarray(inputs["head_W"], np.float32)
    head_b = np.asarray(inputs["head_b"], np.float32)
    steps = int(np.asarray(inputs["steps"]))
    with_bias = bool(np.any(block_b))

    embWT = _tile_k(embed_W.T).astype(np.float16)
    headWT = _tile_k(head_W.T).astype(np.float16)
    Wt = block_W.transpose(0, 2, 1) * np.float32(0.5)
    Wa = Wt.astype(np.float16)
    Wb = (2.0 * Wt - Wa.astype(np.float32)).astype(np.float16)
    Wab = np.stack(
        [
            np.stack([_tile_k(Wa[n]) for n in range(NB)]),
            np.stack([_tile_k(Wb[n]) for n in range(NB)]),
        ]
    )
    embB = np.ascontiguousarray(embed_b.reshape(KH, 128).T)
    bT = np.ascontiguousarray(
        block_b.reshape(NB, KH, 128).transpose(2, 0, 1).reshape(128, NB * KH)
    )
    headB = np.ascontiguousarray(head_b.reshape(KO, 128).T)

    in_maps = []
    for ci in range(NCORES):
        xTl = _tile_k(np.ascontiguousarray(x[ci * BL:(ci + 1) * BL].T)).astype(np.float16)
        in_maps.append(
            dict(xT=xTl, embWT=embWT, embB=embB, Wab=Wab, bT=bT,
                 headWT=headWT, headB=headB)
        )

    nc = build_nc(steps, with_bias)
    res = run_bass_kernel_spmd(nc, in_maps, core_ids=list(range(NCORES)))

    out = np.empty((B, DOUT), np.float32)
    for ci in range(NCORES):
        oT = res.results[ci]["outT"]
        out[ci * BL:(ci + 1) * BL] = (
            oT.reshape(128, KO, BL).transpose(2, 1, 0).reshape(BL, DOUT)
        )
    return out


# revision 7
# speedup vs baseline: 1.0017x; 1.0009x over previous
"""Trainium2 Bass kernel v3: baseline fp16 engine scheme + exact triangular
trim of the (block, step) iteration space.

Trim (exact, zero error):
  - dead-code: v[n] at step s only reaches the head if s <= steps-9+n
  - zero-prop: with block_b == 0, v[n] stays exactly 0 until step n+1
  => 210 of 300 block-steps remain at steps=30 (30% less PE work).

Schedule: per step, active blocks descending, paired into units of <=2.
Pipeline: phase_a(i+1) emitted before phase_b(i); where unit i+1 reads a v
written by unit i, a bubble (None unit) is inserted so emission order stays
correct (Tile gives sequential semantics per tile in emission order).
"""

import numpy as np

import concourse.bass as bass
import concourse.bacc as bacc
import concourse.mybir as mybir
from concourse.bass_utils import run_bass_kernel_spmd
from concourse.tile import TileContext

F32 = mybir.dt.float32
F16 = mybir.dt.float16

B, DIN, H, DOUT, NB = 1024, 512, 1024, 512, 10
NCORES = 8
BL = B // NCORES  # 128
KH = H // 128     # 8
KD = DIN // 128   # 4
KO = DOUT // 128  # 4
INNER = 5
Tanh = mybir.ActivationFunctionType.Tanh
Copy = mybir.ActivationFunctionType.Copy
Ident = mybir.ActivationFunctionType.Identity
MULT = mybir.AluOpType.mult
ADD = mybir.AluOpType.add


def build_units(steps: int, with_bias: bool):
    """Unit list: each entry is a list of (s, n) block-steps (len 1-2) or
    None (pipeline bubble)."""
    units = []
    for s in range(1, steps + 1):
        ns = [n for n in range(NB - 1, -1, -1)
              if s <= steps - (NB - 1) + n and (with_bias or s >= n + 1)]
        for i in range(0, len(ns), 2):
            units.append([(s, n) for n in ns[i:i + 2]])

    def writes(u):
        return {n for (_, n) in u} if u else set()

    def reads(u):
        r = set()
        if u:
            for (_, n) in u:
                r.add(n)
                if n > 0:
                    r.add(n - 1)
        return r

    out = []
    for u in units:
        prev = out[-1] if out else None
        if prev is not None and (writes(prev) & reads(u)):
            out.append(None)
        out.append(u)
    return out


def build_nc(steps: int, with_bias: bool = False):
    nc = bacc.Bacc(None, target_bir_lowering=False)
    xT = nc.dram_tensor("xT", [128, KD * BL], F16, kind="ExternalInput")
    embWT = nc.dram_tensor("embWT", [128, KD * H], F16, kind="ExternalInput")
    embB = nc.dram_tensor("embB", [128, KH], F32, kind="ExternalInput")
    Wab = nc.dram_tensor("Wab", [2, NB, 128, KH * H], F16, kind="ExternalInput")
    bT = nc.dram_tensor("bT", [128, NB * KH], F32, kind="ExternalInput")
    headWT = nc.dram_tensor("headWT", [128, KH * DOUT], F16, kind="ExternalInput")
    headB = nc.dram_tensor("headB", [128, KO], F32, kind="ExternalInput")
    outT = nc.dram_tensor("outT", [128, KO * BL], F32, kind="ExternalOutput")

    units = build_units(steps, with_bias)
    nreal = sum(1 for u in units if u)

    with TileContext(nc) as tc:
        with (
            tc.tile_pool(name="const", bufs=1) as cpool,
            tc.tile_pool(name="state", bufs=1) as spool,
            tc.tile_pool(name="wts", bufs=4) as wpool,
            tc.tile_pool(name="work", bufs=2) as kpool,
            tc.tile_pool(name="psum", bufs=1, space="PSUM") as ppool,
        ):
            xT_sb = cpool.tile([128, KD * BL], F16, tag="xt", bufs=1)
            embWT_sb = cpool.tile([128, KD * H], F16, tag="embwt", bufs=1)
            embB_sb = cpool.tile([128, KH], F32, tag="embb", bufs=1)
            bT_sb = cpool.tile([128, NB * KH], F32, tag="bt", bufs=1)
            headWT_sb = cpool.tile([128, KH * DOUT], F16, tag="hwt", bufs=1)
            headB_sb = cpool.tile([128, KO], F32, tag="hb", bufs=1)

            nc.sync.dma_start(xT_sb[:], xT[:])
            nc.sync.dma_start(embWT_sb[:], embWT[:])
            nc.sync.dma_start(embB_sb[:], embB[:])
            if with_bias:
                nc.sync.dma_start(bT_sb[:], bT[:])

            v = [spool.tile([128, H], F32, tag=f"v{n}", bufs=1, name=f"v{n}")
                 for n in range(NB)]
            xemb = spool.tile([128, H], F32, tag="xemb", bufs=1)
            for n in range(NB):
                if n % 2 == 0:
                    nc.vector.memset(v[n][:], 0.0)
                else:
                    nc.gpsimd.memset(v[n][:], 0.0)

            bfull = None
            if with_bias:
                # fp16 bias tensors (SBUF budget); bias-add error ~2^-11 on c
                bfull = [spool.tile([128, H], F16, tag=f"bf{n}", bufs=1,
                                    name=f"bf{n}") for n in range(NB)]
                zed = spool.tile([128, 128], F32, tag="zed", bufs=1)
                nc.vector.memset(zed[:], 0.0)
                for n in range(NB):
                    for m in range(KH):
                        nc.scalar.activation(
                            bfull[n][:, m * 128:(m + 1) * 128], zed[:], Ident,
                            bias=bT_sb[:, n * KH + m:n * KH + m + 1], scale=0.0)

            # psum banks: [set][pair-position][half]
            pbank = [[[ppool.tile([128, 512], F32, tag=f"ps{s}{p}{h}", bufs=1,
                                  name=f"ps{s}{p}{h}")
                       for h in range(2)] for p in range(2)] for s in range(2)]

            def emit_warm(bank):
                # Set every psum element's has_written bit via a full-bank
                # start=True matmul (values are garbage; prefills replace
                # them). Needed so later start=False matmuls ACCUMULATE onto
                # Act/DVE-prefilled values instead of overwriting them —
                # only TensorE matmuls set has_written.
                nc.tensor.matmul(
                    bank[:], xT_sb[:, 0:128], xT_sb[:, 0:KD * BL],
                    start=True, stop=True, skip_group_check=True,
                )

            def emit_embed():
                for m in range(KH):
                    pe = pbank[1][1][1][:, (m % 4) * 128:(m % 4 + 1) * 128]
                    for k in range(KD):
                        nc.tensor.matmul(
                            pe,
                            embWT_sb[:, k * H + m * 128:k * H + (m + 1) * 128],
                            xT_sb[:, k * BL:(k + 1) * BL],
                            start=(k == 0), stop=(k == KD - 1),
                        )
                    nc.scalar.activation(
                        xemb[:, m * 128:(m + 1) * 128], pe, Ident,
                        bias=embB_sb[:, m:m + 1], scale=1.0,
                    )

            state = {}   # unit idx -> {n: [w, c, vh, u]}
            sets = {}    # unit idx -> psum set

            def emit_phase_a(i, fast=False):
                u_list = units[i]
                if u_list is None:
                    return
                S = sets[i]
                ust = {}
                wb = 3 if with_bias else 4  # SBUF budget in bias mode
                for pi, (s, n) in enumerate(u_list):
                    par = (s + 1) % 2  # s=1 -> Wa (matches baseline order)
                    w = []
                    for k in range(KH):
                        wk = wpool.tile([128, H], F16, tag=f"w{k}", bufs=wb,
                                        name=f"w{k}")
                        nc.sync.dma_start(
                            wk[:], Wab[par, n, :, k * H:(k + 1) * H])
                        w.append(wk)
                    binT = xemb if n == 0 else v[n - 1]
                    c = kpool.tile([128, H], F32, tag="c", bufs=4, name="c")
                    if with_bias:
                        nc.gpsimd.tensor_tensor(c[:], v[n][:], binT[:], ADD)
                        nc.gpsimd.tensor_tensor(c[:], c[:], bfull[n][:], ADD)
                    elif fast:
                        # post-bubble units: c-add is latency-critical; split
                        # halves across DVE (0.54us) and gpsimd (1.02us)
                        nc.vector.scalar_tensor_tensor(
                            c[:, 0:512], v[n][:, 0:512], 1.0,
                            binT[:, 0:512], MULT, ADD)
                        nc.gpsimd.tensor_tensor(
                            c[:, 512:1024], v[n][:, 512:1024],
                            binT[:, 512:1024], ADD)
                    else:
                        nc.gpsimd.tensor_tensor(c[:], v[n][:], binT[:], ADD)
                    vh = kpool.tile([128, H], F32, tag="vh", bufs=4, name="vh")
                    nc.gpsimd.tensor_scalar_mul(vh[:], v[n][:], 0.5)
                    u = kpool.tile([128, H], F16, tag="u", bufs=6, name="u")
                    nc.scalar.activation(u[:], c[:], Tanh, bias=0.0, scale=1.0)
                    nc.scalar.activation(pbank[S][pi][0][:], c[:, 0:512],
                                         Copy, scale=1.0)
                    nc.vector.tensor_copy(pbank[S][pi][1][:], c[:, 512:1024])
                    ust[n] = [w, c, vh, u]
                state[i] = ust

            def emit_phase_b(i):
                u_list = units[i]
                if u_list is None:
                    return
                S = sets[i]
                ust = state.pop(i)
                order = [n for (_, n) in u_list]
                for j in range(1, INNER):
                    last = j == INNER - 1
                    for pi, n in enumerate(order):
                        w, c, vh, u = ust[n]
                        t = kpool.tile([128, H], F16, tag="t", bufs=4,
                                       name="t")
                        for h in range(2):
                            bank = pbank[S][pi][h]
                            for m in range(4):
                                mg = h * 4 + m
                                for k in range(KH):
                                    nc.tensor.matmul(
                                        bank[:, m * 128:(m + 1) * 128],
                                        w[k][:, mg * 128:(mg + 1) * 128],
                                        u[:, k * 128:(k + 1) * 128],
                                        start=False, stop=(k == KH - 1),
                                        skip_group_check=True,
                                    )
                            nc.scalar.activation(
                                t[:, h * 512:(h + 1) * 512], bank[:], Tanh,
                                bias=0.0, scale=1.0)
                            if not last:
                                if h == 0:
                                    nc.scalar.activation(
                                        bank[:], c[:, 0:512], Copy, scale=1.0)
                                else:
                                    nc.vector.tensor_copy(
                                        bank[:], c[:, 512:1024])
                        if not last:
                            un = kpool.tile([128, H], F16, tag="u", bufs=6,
                                            name="un")
                            nc.vector.scalar_tensor_tensor(
                                un[:], u[:], 0.5, t[:], MULT, ADD)
                            ust[n][3] = un
                        else:
                            u5 = kpool.tile([128, H], F16, tag="u5", bufs=2,
                                            name="u5")
                            nc.vector.scalar_tensor_tensor(
                                u5[:], u[:], 0.5, t[:], MULT, ADD)
                            nc.vector.scalar_tensor_tensor(
                                v[n][:], u5[:], 0.25, vh[:], MULT, ADD)

            def emit_head(S):
                nc.sync.dma_start(headWT_sb[:], headWT[:])
                nc.sync.dma_start(headB_sb[:], headB[:])
                outsb = kpool.tile([128, KO * BL], F32, tag="outsb", bufs=1)
                v9h = kpool.tile([128, H], F16, tag="v9h", bufs=1)
                nc.vector.tensor_copy(v9h[:], v[NB - 1][:])
                for m in range(KO):
                    ph = pbank[S][0][0][:, m * 128:(m + 1) * 128]
                    for k in range(KH):
                        nc.tensor.matmul(
                            ph,
                            headWT_sb[:, k * DOUT + m * 128:k * DOUT + (m + 1) * 128],
                            v9h[:, k * 128:(k + 1) * 128],
                            start=(k == 0), stop=(k == KH - 1),
                        )
                    nc.scalar.activation(
                        outsb[:, m * BL:(m + 1) * BL], ph, Ident,
                        bias=headB_sb[:, m:m + 1], scale=1.0,
                    )
                nc.sync.dma_start(outT[:], outsb[:])

            # assign psum sets by real-unit parity
            r = 0
            for i, u_list in enumerate(units):
                if u_list is not None:
                    sets[i] = r % 2
                    r += 1

            for S in range(2):
                for p in range(2):
                    for h in range(2):
                        emit_warm(pbank[S][p][h])
            emit_embed()
            # embed's start=True groups cleared this bank's has_written bits
            # and only re-set the last slice; re-warm before unit use.
            emit_warm(pbank[1][1][1])
            def emit_bubble_fill(i):
                # Keep the PE p-state hot across a pipeline bubble: dummy
                # full-bank matmuls on the next unit's pair-1 banks (its
                # prefill rewrites values; start=True leaves has_written set).
                nxt = units[i + 1] if i + 1 < len(units) else None
                if nxt is None:
                    return
                S = sets[i + 1]
                for r in range(6):
                    emit_warm(pbank[S][1][r % 2])

            if units:
                emit_phase_a(0, fast=True)
            for i in range(len(units)):
                if units[i] is None:
                    emit_bubble_fill(i)
                if i + 1 < len(units):
                    emit_phase_a(i + 1, fast=(units[i] is None))
                emit_phase_b(i)
            emit_head(nreal % 2)

    nc.compile()
    return nc


def _tile_k(a):
    """[K, M] -> [128, (K//128)*M] laid out (k_lo, k_hi, m)."""
    K, M = a.shape
    return np.ascontiguousarray(
        a.reshape(K // 128, 128, M).transpose(1, 0, 2).reshape(128, (K // 128) * M)
    )


def kernel(**inputs) -> np.ndarray:
    x = np.asarray(inputs["x"], np.float32)
    embed_W = np.asarray(inputs["embed_W"], np.float32)
    embed_b = np.asarray(inputs["embed_b"], np.float32)
    block_W = np.asarray(inputs["block_W"], np.float32)
    block_b = np.asarray(inputs["block_b"], np.float32)
    head_W = np.asarray(inputs["head_W"], np.float32)
    head_b = np.asarray(inputs["head_b"], np.float32)
    steps = int(np.asarray(inputs["steps"]))
    with_bias = bool(np.any(block_b))

    embWT = _tile_k(embed_W.T).astype(np.float16)
    headWT = _tile_k(head_W.T).astype(np.float16)
    Wt = block_W.transpose(0, 2, 1) * np.float32(0.5)
    Wa = Wt.astype(np.float16)
    Wb = (2.0 * Wt - Wa.astype(np.float32)).astype(np.float16)
    Wab = np.stack(
        [
            np.stack([_tile_k(Wa[n]) for n in range(NB)]),
            np.stack([_tile_k(Wb[n]) for n in range(NB)]),
        ]
    )
    embB = np.ascontiguousarray(embed_b.reshape(KH, 128).T)
    bT = np.ascontiguousarray(
        block_b.reshape(NB, KH, 128).transpose(2, 0, 1).reshape(128, NB * KH)
    )
    headB = np.ascontiguousarray(head_b.reshape(KO, 128).T)

    in_maps = []
    for ci in range(NCORES):
        xTl = _tile_k(np.ascontiguousarray(x[ci * BL:(ci + 1) * BL].T)).astype(np.float16)
        in_maps.append(
            dict(xT=xTl, embWT=embWT, embB=embB, Wab=Wab, bT=bT,
                 headWT=headWT, headB=headB)
        )

    nc = build_nc(steps, with_bias)
    res = run_bass_kernel_spmd(nc, in_maps, core_ids=list(range(NCORES)))

    out = np.empty((B, DOUT), np.float32)
    for ci in range(NCORES):
        oT = res.results[ci]["outT"]
        out[ci * BL:(ci + 1) * BL] = (
            oT.reshape(128, KO, BL).transpose(2, 1, 0).reshape(BL, DOUT)
        )
    return out


# revision 8
# speedup vs baseline: 1.0018x; 1.0001x over previous
"""Trainium2 Bass kernel v3: baseline fp16 engine scheme + exact triangular
trim of the (block, step) iteration space.

Trim (exact, zero error):
  - dead-code: v[n] at step s only reaches the head if s <= steps-9+n
  - zero-prop: with block_b == 0, v[n] stays exactly 0 until step n+1
  => 210 of 300 block-steps remain at steps=30 (30% less PE work).

Schedule: per step, active blocks descending, paired into units of <=2.
Pipeline: phase_a(i+1) emitted before phase_b(i); where unit i+1 reads a v
written by unit i, a bubble (None unit) is inserted so emission order stays
correct (Tile gives sequential semantics per tile in emission order).
"""

import numpy as np

import concourse.bass as bass
import concourse.bacc as bacc
import concourse.mybir as mybir
from concourse.bass_utils import run_bass_kernel_spmd
from concourse.tile import TileContext

F32 = mybir.dt.float32
F16 = mybir.dt.float16

B, DIN, H, DOUT, NB = 1024, 512, 1024, 512, 10
NCORES = 8
BL = B // NCORES  # 128
KH = H // 128     # 8
KD = DIN // 128   # 4
KO = DOUT // 128  # 4
INNER = 5
Tanh = mybir.ActivationFunctionType.Tanh
Copy = mybir.ActivationFunctionType.Copy
Ident = mybir.ActivationFunctionType.Identity
MULT = mybir.AluOpType.mult
ADD = mybir.AluOpType.add


def build_units(steps: int, with_bias: bool):
    """Unit list: each entry is a list of (s, n) block-steps (len 1-2) or
    None (pipeline bubble)."""
    units = []
    for s in range(1, steps + 1):
        ns = [n for n in range(NB - 1, -1, -1)
              if s <= steps - (NB - 1) + n and (with_bias or s >= n + 1)]
        for i in range(0, len(ns), 2):
            units.append([(s, n) for n in ns[i:i + 2]])

    def writes(u):
        return {n for (_, n) in u} if u else set()

    def reads(u):
        r = set()
        if u:
            for (_, n) in u:
                r.add(n)
                if n > 0:
                    r.add(n - 1)
        return r

    out = []
    for u in units:
        prev = out[-1] if out else None
        if prev is not None and (writes(prev) & reads(u)):
            out.append(None)
        out.append(u)
    return out


def build_nc(steps: int, with_bias: bool = False):
    nc = bacc.Bacc(None, target_bir_lowering=False)
    xT = nc.dram_tensor("xT", [128, KD * BL], F16, kind="ExternalInput")
    embWT = nc.dram_tensor("embWT", [128, KD * H], F16, kind="ExternalInput")
    embB = nc.dram_tensor("embB", [128, KH], F32, kind="ExternalInput")
    Wab = nc.dram_tensor("Wab", [2, NB, 128, KH * H], F16, kind="ExternalInput")
    bT = nc.dram_tensor("bT", [128, NB * KH], F32, kind="ExternalInput")
    headWT = nc.dram_tensor("headWT", [128, KH * DOUT], F16, kind="ExternalInput")
    headB = nc.dram_tensor("headB", [128, KO], F32, kind="ExternalInput")
    outT = nc.dram_tensor("outT", [128, KO * BL], F32, kind="ExternalOutput")

    units = build_units(steps, with_bias)
    nreal = sum(1 for u in units if u)

    with TileContext(nc) as tc:
        with (
            tc.tile_pool(name="const", bufs=1) as cpool,
            tc.tile_pool(name="state", bufs=1) as spool,
            tc.tile_pool(name="wts", bufs=4) as wpool,
            tc.tile_pool(name="work", bufs=2) as kpool,
            tc.tile_pool(name="psum", bufs=1, space="PSUM") as ppool,
        ):
            xT_sb = cpool.tile([128, KD * BL], F16, tag="xt", bufs=1)
            embWT_sb = cpool.tile([128, KD * H], F16, tag="embwt", bufs=1)
            embB_sb = cpool.tile([128, KH], F32, tag="embb", bufs=1)
            bT_sb = cpool.tile([128, NB * KH], F32, tag="bt", bufs=1)
            headWT_sb = cpool.tile([128, KH * DOUT], F16, tag="hwt", bufs=1)
            headB_sb = cpool.tile([128, KO], F32, tag="hb", bufs=1)

            nc.sync.dma_start(xT_sb[:], xT[:])
            nc.sync.dma_start(embWT_sb[:], embWT[:])
            nc.sync.dma_start(embB_sb[:], embB[:])
            if with_bias:
                nc.sync.dma_start(bT_sb[:], bT[:])

            v = [spool.tile([128, H], F32, tag=f"v{n}", bufs=1, name=f"v{n}")
                 for n in range(NB)]
            xemb = spool.tile([128, H], F32, tag="xemb", bufs=1)
            for n in range(NB):
                if n % 2 == 0:
                    nc.vector.memset(v[n][:], 0.0)
                else:
                    nc.gpsimd.memset(v[n][:], 0.0)

            bfull = None
            if with_bias:
                # fp16 bias tensors (SBUF budget); bias-add error ~2^-11 on c
                bfull = [spool.tile([128, H], F16, tag=f"bf{n}", bufs=1,
                                    name=f"bf{n}") for n in range(NB)]
                zed = spool.tile([128, 128], F32, tag="zed", bufs=1)
                nc.vector.memset(zed[:], 0.0)
                for n in range(NB):
                    for m in range(KH):
                        nc.scalar.activation(
                            bfull[n][:, m * 128:(m + 1) * 128], zed[:], Ident,
                            bias=bT_sb[:, n * KH + m:n * KH + m + 1], scale=0.0)

            # psum banks: [set][pair-position][half]
            pbank = [[[ppool.tile([128, 512], F32, tag=f"ps{s}{p}{h}", bufs=1,
                                  name=f"ps{s}{p}{h}")
                       for h in range(2)] for p in range(2)] for s in range(2)]

            def emit_warm(bank):
                # Set every psum element's has_written bit via a full-bank
                # start=True matmul (values are garbage; prefills replace
                # them). Needed so later start=False matmuls ACCUMULATE onto
                # Act/DVE-prefilled values instead of overwriting them —
                # only TensorE matmuls set has_written.
                nc.tensor.matmul(
                    bank[:], xT_sb[:, 0:128], xT_sb[:, 0:KD * BL],
                    start=True, stop=True, skip_group_check=True,
                )

            def emit_embed():
                for m in range(KH):
                    pe = pbank[1][1][1][:, (m % 4) * 128:(m % 4 + 1) * 128]
                    for k in range(KD):
                        nc.tensor.matmul(
                            pe,
                            embWT_sb[:, k * H + m * 128:k * H + (m + 1) * 128],
                            xT_sb[:, k * BL:(k + 1) * BL],
                            start=(k == 0), stop=(k == KD - 1),
                        )
                    nc.scalar.activation(
                        xemb[:, m * 128:(m + 1) * 128], pe, Ident,
                        bias=embB_sb[:, m:m + 1], scale=1.0,
                    )

            state = {}   # unit idx -> {n: [w, c, vh, u]}
            sets = {}    # unit idx -> psum set

            def emit_phase_a(i, fast=False):
                u_list = units[i]
                if u_list is None:
                    return
                S = sets[i]
                ust = {}
                wb = 3 if with_bias else 4  # SBUF budget in bias mode
                for pi, (s, n) in enumerate(u_list):
                    par = (s + 1) % 2  # s=1 -> Wa (matches baseline order)
                    w = []
                    for k in range(KH):
                        wk = wpool.tile([128, H], F16, tag=f"w{k}", bufs=wb,
                                        name=f"w{k}")
                        nc.sync.dma_start(
                            wk[:], Wab[par, n, :, k * H:(k + 1) * H])
                        w.append(wk)
                    binT = xemb if n == 0 else v[n - 1]
                    c = kpool.tile([128, H], F32, tag="c", bufs=4, name="c")
                    if with_bias:
                        nc.gpsimd.tensor_tensor(c[:], v[n][:], binT[:], ADD)
                        nc.gpsimd.tensor_tensor(c[:], c[:], bfull[n][:], ADD)
                    elif fast:
                        # post-bubble units: c-add is latency-critical; split
                        # halves across DVE (0.54us) and gpsimd (1.02us)
                        nc.vector.scalar_tensor_tensor(
                            c[:, 0:512], v[n][:, 0:512], 1.0,
                            binT[:, 0:512], MULT, ADD)
                        nc.gpsimd.tensor_tensor(
                            c[:, 512:1024], v[n][:, 512:1024],
                            binT[:, 512:1024], ADD)
                    else:
                        nc.gpsimd.tensor_tensor(c[:], v[n][:], binT[:], ADD)
                    vh = kpool.tile([128, H], F32, tag="vh", bufs=4, name="vh")
                    nc.gpsimd.tensor_scalar_mul(vh[:], v[n][:], 0.5)
                    u = kpool.tile([128, H], F16, tag="u", bufs=6, name="u")
                    if fast:
                        # latency-ordered Act FIFO: bank-0 operands (u half 0
                        # + prefill 0) complete before c half 1 is even needed
                        nc.scalar.activation(u[:, 0:512], c[:, 0:512], Tanh,
                                             bias=0.0, scale=1.0)
                        nc.scalar.activation(pbank[S][pi][0][:], c[:, 0:512],
                                             Copy, scale=1.0)
                        nc.scalar.activation(u[:, 512:1024], c[:, 512:1024],
                                             Tanh, bias=0.0, scale=1.0)
                    else:
                        nc.scalar.activation(u[:], c[:], Tanh, bias=0.0,
                                             scale=1.0)
                        nc.scalar.activation(pbank[S][pi][0][:], c[:, 0:512],
                                             Copy, scale=1.0)
                    nc.vector.tensor_copy(pbank[S][pi][1][:], c[:, 512:1024])
                    ust[n] = [w, c, vh, u]
                state[i] = ust

            def emit_phase_b(i):
                u_list = units[i]
                if u_list is None:
                    return
                S = sets[i]
                ust = state.pop(i)
                order = [n for (_, n) in u_list]
                for j in range(1, INNER):
                    last = j == INNER - 1
                    for pi, n in enumerate(order):
                        w, c, vh, u = ust[n]
                        t = kpool.tile([128, H], F16, tag="t", bufs=4,
                                       name="t")
                        for h in range(2):
                            bank = pbank[S][pi][h]
                            for m in range(4):
                                mg = h * 4 + m
                                for k in range(KH):
                                    nc.tensor.matmul(
                                        bank[:, m * 128:(m + 1) * 128],
                                        w[k][:, mg * 128:(mg + 1) * 128],
                                        u[:, k * 128:(k + 1) * 128],
                                        start=False, stop=(k == KH - 1),
                                        skip_group_check=True,
                                    )
                            nc.scalar.activation(
                                t[:, h * 512:(h + 1) * 512], bank[:], Tanh,
                                bias=0.0, scale=1.0)
                            if not last:
                                if h == 0:
                                    nc.scalar.activation(
                                        bank[:], c[:, 0:512], Copy, scale=1.0)
                                else:
                                    nc.vector.tensor_copy(
                                        bank[:], c[:, 512:1024])
                        if not last:
                            un = kpool.tile([128, H], F16, tag="u", bufs=6,
                                            name="un")
                            nc.vector.scalar_tensor_tensor(
                                un[:], u[:], 0.5, t[:], MULT, ADD)
                            ust[n][3] = un
                        else:
                            u5 = kpool.tile([128, H], F16, tag="u5", bufs=2,
                                            name="u5")
                            nc.vector.scalar_tensor_tensor(
                                u5[:], u[:], 0.5, t[:], MULT, ADD)
                            nc.vector.scalar_tensor_tensor(
                                v[n][:], u5[:], 0.25, vh[:], MULT, ADD)

            def emit_head(S):
                nc.sync.dma_start(headWT_sb[:], headWT[:])
                nc.sync.dma_start(headB_sb[:], headB[:])
                outsb = kpool.tile([128, KO * BL], F32, tag="outsb", bufs=1)
                v9h = kpool.tile([128, H], F16, tag="v9h", bufs=1)
                nc.vector.tensor_copy(v9h[:], v[NB - 1][:])
                for m in range(KO):
                    ph = pbank[S][0][0][:, m * 128:(m + 1) * 128]
                    for k in range(KH):
                        nc.tensor.matmul(
                            ph,
                            headWT_sb[:, k * DOUT + m * 128:k * DOUT + (m + 1) * 128],
                            v9h[:, k * 128:(k + 1) * 128],
                            start=(k == 0), stop=(k == KH - 1),
                        )
                    nc.scalar.activation(
                        outsb[:, m * BL:(m + 1) * BL], ph, Ident,
                        bias=headB_sb[:, m:m + 1], scale=1.0,
                    )
                nc.sync.dma_start(outT[:], outsb[:])

            # assign psum sets by real-unit parity
            r = 0
            for i, u_list in enumerate(units):
                if u_list is not None:
                    sets[i] = r % 2
                    r += 1

            for S in range(2):
                for p in range(2):
                    for h in range(2):
                        emit_warm(pbank[S][p][h])
            emit_embed()
            # embed's start=True groups cleared this bank's has_written bits
            # and only re-set the last slice; re-warm before unit use.
            emit_warm(pbank[1][1][1])
            def emit_bubble_fill(i):
                # Keep the PE p-state hot across a pipeline bubble: dummy
                # full-bank matmuls on the next unit's pair-1 banks (its
                # prefill rewrites values; start=True leaves has_written set).
                nxt = units[i + 1] if i + 1 < len(units) else None
                if nxt is None:
                    return
                S = sets[i + 1]
                for r in range(6):
                    emit_warm(pbank[S][1][r % 2])

            if units:
                emit_phase_a(0, fast=True)
            for i in range(len(units)):
                if units[i] is None:
                    emit_bubble_fill(i)
                if i + 1 < len(units):
                    emit_phase_a(i + 1, fast=(units[i] is None))
                emit_phase_b(i)
            emit_head(nreal % 2)

    nc.compile()
    return nc


def _tile_k(a):
    """[K, M] -> [128, (K//128)*M] laid out (k_lo, k_hi, m)."""
    K, M = a.shape
    return np.ascontiguousarray(
        a.reshape(K // 128, 128, M).transpose(1, 0, 2).reshape(128, (K // 128) * M)
    )


def kernel(**inputs) -> np.ndarray:
    x = np.asarray(inputs["x"], np.float32)
    embed_W = np.asarray(inputs["embed_W"], np.float32)
    embed_b = np.asarray(inputs["embed_b"], np.float32)
    block_W = np.asarray(inputs["block_W"], np.float32)
    block_b = np.asarray(inputs["block_b"], np.float32)
    head_W = np.asarray(inputs["head_W"], np.float32)
    head_b = np.asarray(inputs["head_b"], np.float32)
    steps = int(np.asarray(inputs["steps"]))
    with_bias = bool(np.any(block_b))

    embWT = _tile_k(embed_W.T).astype(np.float16)
    headWT = _tile_k(head_W.T).astype(np.float16)
    Wt = block_W.transpose(0, 2, 1) * np.float32(0.5)
    Wa = Wt.astype(np.float16)
    Wb = (2.0 * Wt - Wa.astype(np.float32)).astype(np.float16)
    Wab = np.stack(
        [
            np.stack([_tile_k(Wa[n]) for n in range(NB)]),
            np.stack([_tile_k(Wb[n]) for n in range(NB)]),
        ]
    )
    embB = np.ascontiguousarray(embed_b.reshape(KH, 128).T)
    bT = np.ascontiguousarray(
        block_b.reshape(NB, KH, 128).transpose(2, 0, 1).reshape(128, NB * KH)
    )
    headB = np.ascontiguousarray(head_b.reshape(KO, 128).T)

    in_maps = []
    for ci in range(NCORES):
        xTl = _tile_k(np.ascontiguousarray(x[ci * BL:(ci + 1) * BL].T)).astype(np.float16)
        in_maps.append(
            dict(xT=xTl, embWT=embWT, embB=embB, Wab=Wab, bT=bT,
                 headWT=headWT, headB=headB)
        )

    nc = build_nc(steps, with_bias)
    res = run_bass_kernel_spmd(nc, in_maps, core_ids=list(range(NCORES)))

    out = np.empty((B, DOUT), np.float32)
    for ci in range(NCORES):
        oT = res.results[ci]["outT"]
        out[ci * BL:(ci + 1) * BL] = (
            oT.reshape(128, KO, BL).transpose(2, 1, 0).reshape(BL, DOUT)
        )
    return out


# revision 9
# speedup vs baseline: 1.0103x; 1.0085x over previous
"""Trainium2 Bass kernel v3: baseline fp16 engine scheme + exact triangular
trim of the (block, step) iteration space.

Trim (exact, zero error):
  - dead-code: v[n] at step s only reaches the head if s <= steps-9+n
  - zero-prop: with block_b == 0, v[n] stays exactly 0 until step n+1
  => 210 of 300 block-steps remain at steps=30 (30% less PE work).

Schedule: per step, active blocks descending, paired into units of <=2.
Pipeline: phase_a(i+1) emitted before phase_b(i); where unit i+1 reads a v
written by unit i, a bubble (None unit) is inserted so emission order stays
correct (Tile gives sequential semantics per tile in emission order).
"""

import numpy as np

import concourse.bass as bass
import concourse.bacc as bacc
import concourse.mybir as mybir
from concourse.bass_utils import run_bass_kernel_spmd
from concourse.tile import TileContext

F32 = mybir.dt.float32
F16 = mybir.dt.float16

B, DIN, H, DOUT, NB = 1024, 512, 1024, 512, 10
NCORES = 8
BL = B // NCORES  # 128
KH = H // 128     # 8
KD = DIN // 128   # 4
KO = DOUT // 128  # 4
INNER = 5
Tanh = mybir.ActivationFunctionType.Tanh
Copy = mybir.ActivationFunctionType.Copy
Ident = mybir.ActivationFunctionType.Identity
MULT = mybir.AluOpType.mult
ADD = mybir.AluOpType.add


def build_units(steps: int, with_bias: bool):
    """Unit list: each entry is a list of (s, n) block-steps (len 1-2) or
    None (pipeline bubble)."""
    units = []
    for s in range(1, steps + 1):
        ns = [n for n in range(NB - 1, -1, -1)
              if s <= steps - (NB - 1) + n and (with_bias or s >= n + 1)]
        for i in range(0, len(ns), 2):
            units.append([(s, n) for n in ns[i:i + 2]])

    def writes(u):
        return {n for (_, n) in u} if u else set()

    def reads(u):
        r = set()
        if u:
            for (_, n) in u:
                r.add(n)
                if n > 0:
                    r.add(n - 1)
        return r

    out = []
    for u in units:
        prev = out[-1] if out else None
        if prev is not None and (writes(prev) & reads(u)):
            out.append(None)
        out.append(u)
    return out


def build_nc(steps: int, with_bias: bool = False):
    nc = bacc.Bacc(None, target_bir_lowering=False)
    xT = nc.dram_tensor("xT", [128, KD * BL], F16, kind="ExternalInput")
    embWT = nc.dram_tensor("embWT", [128, KD * H], F16, kind="ExternalInput")
    embB = nc.dram_tensor("embB", [128, KH], F32, kind="ExternalInput")
    Wab = nc.dram_tensor("Wab", [2, NB, 128, KH * H], F16, kind="ExternalInput")
    bT = nc.dram_tensor("bT", [128, NB * KH], F32, kind="ExternalInput")
    headWT = nc.dram_tensor("headWT", [128, KH * DOUT], F16, kind="ExternalInput")
    headB = nc.dram_tensor("headB", [128, KO], F32, kind="ExternalInput")
    outT = nc.dram_tensor("outT", [128, KO * BL], F32, kind="ExternalOutput")

    units = build_units(steps, with_bias)
    nreal = sum(1 for u in units if u)

    with TileContext(nc) as tc:
        with (
            tc.tile_pool(name="const", bufs=1) as cpool,
            tc.tile_pool(name="state", bufs=1) as spool,
            tc.tile_pool(name="wts", bufs=4) as wpool,
            tc.tile_pool(name="work", bufs=2) as kpool,
            tc.tile_pool(name="psum", bufs=1, space="PSUM") as ppool,
        ):
            xT_sb = cpool.tile([128, KD * BL], F16, tag="xt", bufs=1)
            embWT_sb = cpool.tile([128, KD * H], F16, tag="embwt", bufs=1)
            embB_sb = cpool.tile([128, KH], F32, tag="embb", bufs=1)
            bT_sb = cpool.tile([128, NB * KH], F32, tag="bt", bufs=1)
            headWT_sb = cpool.tile([128, KH * DOUT], F16, tag="hwt", bufs=1)
            headB_sb = cpool.tile([128, KO], F32, tag="hb", bufs=1)

            nc.sync.dma_start(xT_sb[:], xT[:])
            nc.sync.dma_start(embWT_sb[:], embWT[:])
            nc.sync.dma_start(embB_sb[:], embB[:])
            if with_bias:
                nc.sync.dma_start(bT_sb[:], bT[:])

            v = [spool.tile([128, H], F32, tag=f"v{n}", bufs=1, name=f"v{n}")
                 for n in range(NB)]
            xemb = spool.tile([128, H], F32, tag="xemb", bufs=1)
            for n in range(NB):
                if n % 2 == 0:
                    nc.vector.memset(v[n][:], 0.0)
                else:
                    nc.gpsimd.memset(v[n][:], 0.0)

            bfull = None
            if with_bias:
                # fp16 bias tensors (SBUF budget); bias-add error ~2^-11 on c
                bfull = [spool.tile([128, H], F16, tag=f"bf{n}", bufs=1,
                                    name=f"bf{n}") for n in range(NB)]
                zed = spool.tile([128, 128], F32, tag="zed", bufs=1)
                nc.vector.memset(zed[:], 0.0)
                for n in range(NB):
                    for m in range(KH):
                        nc.scalar.activation(
                            bfull[n][:, m * 128:(m + 1) * 128], zed[:], Ident,
                            bias=bT_sb[:, n * KH + m:n * KH + m + 1], scale=0.0)

            # psum banks: [set][pair-position][half]
            pbank = [[[ppool.tile([128, 512], F32, tag=f"ps{s}{p}{h}", bufs=1,
                                  name=f"ps{s}{p}{h}")
                       for h in range(2)] for p in range(2)] for s in range(2)]

            def emit_warm(bank):
                # Set every psum element's has_written bit via a full-bank
                # start=True matmul (values are garbage; prefills replace
                # them). Needed so later start=False matmuls ACCUMULATE onto
                # Act/DVE-prefilled values instead of overwriting them —
                # only TensorE matmuls set has_written.
                nc.tensor.matmul(
                    bank[:], xT_sb[:, 0:128], xT_sb[:, 0:KD * BL],
                    start=True, stop=True, skip_group_check=True,
                )

            def emit_embed():
                for m in range(KH):
                    pe = pbank[1][1][1][:, (m % 4) * 128:(m % 4 + 1) * 128]
                    for k in range(KD):
                        nc.tensor.matmul(
                            pe,
                            embWT_sb[:, k * H + m * 128:k * H + (m + 1) * 128],
                            xT_sb[:, k * BL:(k + 1) * BL],
                            start=(k == 0), stop=(k == KD - 1),
                        )
                    nc.scalar.activation(
                        xemb[:, m * 128:(m + 1) * 128], pe, Ident,
                        bias=embB_sb[:, m:m + 1], scale=1.0,
                    )

            state = {}   # unit idx -> {n: [w, c, vh, u]}
            sets = {}    # unit idx -> psum set

            def emit_phase_a(i, fast=False):
                u_list = units[i]
                if u_list is None:
                    return
                S = sets[i]
                ust = {}
                wb = 3 if with_bias else 4  # SBUF budget in bias mode
                for pi, (s, n) in enumerate(u_list):
                    par = (s + 1) % 2  # s=1 -> Wa (matches baseline order)
                    w = []
                    for k in range(KH):
                        wk = wpool.tile([128, H], F16, tag=f"w{k}", bufs=wb,
                                        name=f"w{k}")
                        nc.sync.dma_start(
                            wk[:], Wab[par, n, :, k * H:(k + 1) * H])
                        w.append(wk)
                    binT = xemb if n == 0 else v[n - 1]
                    c = kpool.tile([128, H], F32, tag="c", bufs=(4 if with_bias else 5), name="c")
                    if with_bias:
                        nc.gpsimd.tensor_tensor(c[:], v[n][:], binT[:], ADD)
                        nc.gpsimd.tensor_tensor(c[:], c[:], bfull[n][:], ADD)
                    elif fast:
                        # post-bubble units: c-add is latency-critical; split
                        # halves across DVE (0.54us) and gpsimd (1.02us)
                        nc.vector.scalar_tensor_tensor(
                            c[:, 0:512], v[n][:, 0:512], 1.0,
                            binT[:, 0:512], MULT, ADD)
                        nc.gpsimd.tensor_tensor(
                            c[:, 512:1024], v[n][:, 512:1024],
                            binT[:, 512:1024], ADD)
                    else:
                        nc.gpsimd.tensor_tensor(c[:], v[n][:], binT[:], ADD)
                    vh = kpool.tile([128, H], F32, tag="vh", bufs=(4 if with_bias else 6), name="vh")
                    nc.gpsimd.tensor_scalar_mul(vh[:], v[n][:], 0.5)
                    u = kpool.tile([128, H], F16, tag="u", bufs=6, name="u")
                    if fast:
                        # latency-ordered Act FIFO: bank-0 operands (u half 0
                        # + prefill 0) complete before c half 1 is even needed
                        nc.scalar.activation(u[:, 0:512], c[:, 0:512], Tanh,
                                             bias=0.0, scale=1.0)
                        nc.scalar.activation(pbank[S][pi][0][:], c[:, 0:512],
                                             Copy, scale=1.0)
                        nc.scalar.activation(u[:, 512:1024], c[:, 512:1024],
                                             Tanh, bias=0.0, scale=1.0)
                    else:
                        nc.scalar.activation(u[:], c[:], Tanh, bias=0.0,
                                             scale=1.0)
                        nc.scalar.activation(pbank[S][pi][0][:], c[:, 0:512],
                                             Copy, scale=1.0)
                    nc.vector.tensor_copy(pbank[S][pi][1][:], c[:, 512:1024])
                    ust[n] = [w, c, vh, u]
                state[i] = ust

            def emit_phase_b(i):
                u_list = units[i]
                if u_list is None:
                    return
                S = sets[i]
                ust = state.pop(i)
                order = [n for (_, n) in u_list]
                for j in range(1, INNER):
                    last = j == INNER - 1
                    for pi, n in enumerate(order):
                        w, c, vh, u = ust[n]
                        t = kpool.tile([128, H], F16, tag="t", bufs=4,
                                       name="t")
                        for h in range(2):
                            bank = pbank[S][pi][h]
                            for m in range(4):
                                mg = h * 4 + m
                                for k in range(KH):
                                    nc.tensor.matmul(
                                        bank[:, m * 128:(m + 1) * 128],
                                        w[k][:, mg * 128:(mg + 1) * 128],
                                        u[:, k * 128:(k + 1) * 128],
                                        start=False, stop=(k == KH - 1),
                                        skip_group_check=True,
                                    )
                            nc.scalar.activation(
                                t[:, h * 512:(h + 1) * 512], bank[:], Tanh,
                                bias=0.0, scale=1.0)
                            if not last:
                                if h == 0:
                                    nc.scalar.activation(
                                        bank[:], c[:, 0:512], Copy, scale=1.0)
                                else:
                                    nc.vector.tensor_copy(
                                        bank[:], c[:, 512:1024])
                        if not last:
                            un = kpool.tile([128, H], F16, tag="u", bufs=6,
                                            name="un")
                            nc.vector.scalar_tensor_tensor(
                                un[:], u[:], 0.5, t[:], MULT, ADD)
                            ust[n][3] = un
                        else:
                            u5 = kpool.tile([128, H], F16, tag="u5", bufs=2,
                                            name="u5")
                            nc.vector.scalar_tensor_tensor(
                                u5[:], u[:], 0.5, t[:], MULT, ADD)
                            nc.vector.scalar_tensor_tensor(
                                v[n][:], u5[:], 0.25, vh[:], MULT, ADD)

            def emit_head(S):
                nc.sync.dma_start(headWT_sb[:], headWT[:])
                nc.sync.dma_start(headB_sb[:], headB[:])
                outsb = kpool.tile([128, KO * BL], F32, tag="outsb", bufs=1)
                v9h = kpool.tile([128, H], F16, tag="v9h", bufs=1)
                nc.vector.tensor_copy(v9h[:], v[NB - 1][:])
                for m in range(KO):
                    ph = pbank[S][0][0][:, m * 128:(m + 1) * 128]
                    for k in range(KH):
                        nc.tensor.matmul(
                            ph,
                            headWT_sb[:, k * DOUT + m * 128:k * DOUT + (m + 1) * 128],
                            v9h[:, k * 128:(k + 1) * 128],
                            start=(k == 0), stop=(k == KH - 1),
                        )
                    nc.scalar.activation(
                        outsb[:, m * BL:(m + 1) * BL], ph, Ident,
                        bias=headB_sb[:, m:m + 1], scale=1.0,
                    )
                nc.sync.dma_start(outT[:], outsb[:])

            # assign psum sets by real-unit parity
            r = 0
            for i, u_list in enumerate(units):
                if u_list is not None:
                    sets[i] = r % 2
                    r += 1

            for S in range(2):
                for p in range(2):
                    for h in range(2):
                        emit_warm(pbank[S][p][h])
            emit_embed()
            # embed's start=True groups cleared this bank's has_written bits
            # and only re-set the last slice; re-warm before unit use.
            emit_warm(pbank[1][1][1])
            def emit_bubble_fill(i):
                # Keep the PE p-state hot across a pipeline bubble: dummy
                # full-bank matmuls on the next unit's pair-1 banks (its
                # prefill rewrites values; start=True leaves has_written set).
                nxt = units[i + 1] if i + 1 < len(units) else None
                if nxt is None:
                    return
                S = sets[i + 1]
                for r in range(8):
                    emit_warm(pbank[S][1][r % 2])

            if units:
                emit_phase_a(0, fast=True)
            for i in range(len(units)):
                if units[i] is None:
                    emit_bubble_fill(i)
                if i + 1 < len(units):
                    emit_phase_a(i + 1, fast=(units[i] is None))
                emit_phase_b(i)
            emit_head(nreal % 2)

    nc.compile()
    return nc


def _tile_k(a):
    """[K, M] -> [128, (K//128)*M] laid out (k_lo, k_hi, m)."""
    K, M = a.shape
    return np.ascontiguousarray(
        a.reshape(K // 128, 128, M).transpose(1, 0, 2).reshape(128, (K // 128) * M)
    )


def kernel(**inputs) -> np.ndarray:
    x = np.asarray(inputs["x"], np.float32)
    embed_W = np.asarray(inputs["embed_W"], np.float32)
    embed_b = np.asarray(inputs["embed_b"], np.float32)
    block_W = np.asarray(inputs["block_W"], np.float32)
    block_b = np.asarray(inputs["block_b"], np.float32)
    head_W = np.asarray(inputs["head_W"], np.float32)
    head_b = np.asarray(inputs["head_b"], np.float32)
    steps = int(np.asarray(inputs["steps"]))
    with_bias = bool(np.any(block_b))

    embWT = _tile_k(embed_W.T).astype(np.float16)
    headWT = _tile_k(head_W.T).astype(np.float16)
    Wt = block_W.transpose(0, 2, 1) * np.float32(0.5)
    Wa = Wt.astype(np.float16)
    Wb = (2.0 * Wt - Wa.astype(np.float32)).astype(np.float16)
    Wab = np.stack(
        [
            np.stack([_tile_k(Wa[n]) for n in range(NB)]),
            np.stack([_tile_k(Wb[n]) for n in range(NB)]),
        ]
    )
    embB = np.ascontiguousarray(embed_b.reshape(KH, 128).T)
    bT = np.ascontiguousarray(
        block_b.reshape(NB, KH, 128).transpose(2, 0, 1).reshape(128, NB * KH)
    )
    headB = np.ascontiguousarray(head_b.reshape(KO, 128).T)

    in_maps = []
    for ci in range(NCORES):
        xTl = _tile_k(np.ascontiguousarray(x[ci * BL:(ci + 1) * BL].T)).astype(np.float16)
        in_maps.append(
            dict(xT=xTl, embWT=embWT, embB=embB, Wab=Wab, bT=bT,
                 headWT=headWT, headB=headB)
        )

    nc = build_nc(steps, with_bias)
    res = run_bass_kernel_spmd(nc, in_maps, core_ids=list(range(NCORES)))

    out = np.empty((B, DOUT), np.float32)
    for ci in range(NCORES):
        oT = res.results[ci]["outT"]
        out[ci * BL:(ci + 1) * BL] = (
            oT.reshape(128, KO, BL).transpose(2, 1, 0).reshape(BL, DOUT)
        )
    return out
